# revision 64
# baseline (speedup 1.0000x reference)
"""HDModel retrieval kernel for 8x TRN2 NeuronCores.

reference:
    sims  = l2norm(hvs) @ l2norm(am).T        # [N, C] cosine sims
    preds = argmax(sims, axis=1)              # int32 [N]
    eta   = (sims[:,1]-sims[:,0])*0.25 + 0.5  # f32 [N]

Strategy (data-parallel over N, am replicated — no cross-core comms):
  - Host pre-transposes hvs -> hvsT [D, N/8] per shard and am -> amT [D, C]
    (layout staging only; all math happens on device).
  - sims via f32r (tf32) matmuls, 1 cyc/row on the PE at ap>=256. The
    PE floor is 16*64*512cyc ~ 218.5us/core; this kernel runs ~86% PE
    occupancy (vs 72% effective for the previous 308.9us version).
  - Each 128-row n-tile is computed as TWO half-C passes (classes 0:512
    then 512:1024) so one pass needs only ONE PSUM bank. During the am
    load window a 4-tile "wave" accumulates chunk-paced as am half-
    chunks arrive (hx DMA'd in quarters, interleaved), hiding the am
    DMA behind sims matmuls; the previous version idled the PE ~50us
    in a serial am-load + am-norm prologue.
  - am column norms: DVE/ACT squares each am chunk (bf16);
    reversed-operand matmuls (stationary=sq chunk, moving=ones, ap=1
    => ~0 PE engine time) accumulate per-class norm^2 in column layout
    [c,1]; transpose matmuls vs identity + an exact fp32 ones
    outer-product broadcast 1/max(norm,eps) to [128, C].
  - hvs row norms (needed only for eta): ACT squares hx pieces (bf16),
    reversed ones-matmuls accumulate norm^2 per tile, ~0 PE time.
  - All those norm accumulators share ONE PSUM bank; since opening a
    multi-matmul accumulation group (start=True) resets the whole bank
    row, the bank is memset once and every norm matmul uses
    start=False (see skip_group_check).
  - argmax: DVE max + max_index per half (first-max like jnp), then a
    cross-half compare (strict > keeps the lower index on ties).
  - preds/eta accumulate in [128, NT] tiles; host reorders
    ([p, t] -> n = t*128+p).

This walrus build encodes ONE sync wait per TPB instruction; Tile attaches
several, so a post-pass splits multi-wait instructions into single-wait
same-engine NoOps (see _split_multiwait).
"""
import numpy as np
from contextlib import ExitStack

import concourse.bass as bass
import concourse.mybir as mybir
import concourse.tile as tile
from concourse.bass_utils import run_bass_kernel_spmd

f32 = mybir.dt.float32
f32r = mybir.dt.float32r
bf16 = mybir.dt.bfloat16
u32 = mybir.dt.uint32

N_CORES = 8
N_FULL, D, C = 16384, 4096, 1024
NS = N_FULL // N_CORES          # 2048 rows per core
NT = NS // 128                  # 16 n-tiles
DCH = D // 128                  # 32 d-chunks
CH = C // 2                     # 512 classes per half
WAVE = 4                        # tiles processed chunk-paced during am load
EPS = 1e-8


def _split_multiwait(nc):
    """Split multi-wait instructions into single-wait NoOps (walrus limit)."""
    ctr = [0]

    def mk_nop(engine, wait=None, update=None):
        ctr[0] += 1
        nop = mybir.InstNoOp(name=f"mwsplit_{ctr[0]}", ins=[], outs=[])
        nop.engine = engine
        nop.sync_info = mybir.SyncInfo(
            on_wait=[wait] if wait is not None else [],
            on_update=[update] if update is not None else [],
        )
        return nop

    for f in nc.m.functions:
        for bb in f.blocks:
            new = []
            changed = False
            for inst in bb.instructions:
                si = inst.sync_info
                if si is None:
                    new.append(inst)
                    continue
                waits = list(si.on_wait)
                updates = list(si.on_update)
                pre, post = [], []
                if len(waits) > 1:
                    pre = [mk_nop(inst.engine, wait=w) for w in waits[:-1]]
                    waits = waits[-1:]
                if len(updates) > 1 and type(inst).__name__ != "InstDMACopy":
                    post = [mk_nop(inst.engine, update=u) for u in updates[1:]]
                    updates = updates[:1]
                if pre or post:
                    inst.sync_info = mybir.SyncInfo(on_wait=waits, on_update=updates)
                    new.extend(pre)
                    new.append(inst)
                    new.extend(post)
                    changed = True
                else:
                    new.append(inst)
            if changed:
                bb.instructions = new


def build_nc():
    nc = bass.Bass()
    hvsT = nc.declare_dram_parameter("hvsT", [D, NS], f32r, isOutput=False)
    amT = nc.declare_dram_parameter("amT", [D, C], f32r, isOutput=False)
    ones_b = nc.declare_dram_parameter("ones_b", [128, 1], bf16, isOutput=False)
    ones_r = nc.declare_dram_parameter("ones_r", [1, 128], f32, isOutput=False)
    ident = nc.declare_dram_parameter("ident", [128, 128], f32, isOutput=False)
    preds_o = nc.declare_dram_parameter("preds", [128, NT], u32, isOutput=True)
    eta_o = nc.declare_dram_parameter("eta", [128, NT], f32, isOutput=True)

    with tile.TileContext(nc) as tc, ExitStack() as ctx:
        const_p = ctx.enter_context(tc.tile_pool(name="const", bufs=1))
        am_p = ctx.enter_context(tc.tile_pool(name="am", bufs=1))
        hx_p = ctx.enter_context(tc.tile_pool(name="hx", bufs=4))
        sqa_p = ctx.enter_context(tc.tile_pool(name="sqa", bufs=2))
        sqh_p = ctx.enter_context(tc.tile_pool(name="sqh", bufs=4))
        sc_p = ctx.enter_context(tc.tile_pool(name="sc", bufs=1))
        ep_p = ctx.enter_context(tc.tile_pool(name="ep", bufs=2))
        acc_p = ctx.enter_context(tc.tile_pool(name="acc", bufs=1))
        wv_p = ctx.enter_context(tc.tile_pool(name="wv", bufs=6, space="PSUM"))
        nrm_p = ctx.enter_context(tc.tile_pool(name="nrm", bufs=1, space="PSUM"))
        trbc_p = ctx.enter_context(tc.tile_pool(name="trbc", bufs=1,
                                                space="PSUM"))

        # ---- constants (DMA'd after the first payload transfers so they
        # don't delay the critical hx0/amA0 prefix; ones_ct/ident_t are only
        # needed by am_norm_finalize and go out mid-window) ----
        ones_t = const_p.tile([128, 1], bf16)
        ones_ct = const_p.tile([1, 128], f32)
        ident_t = const_p.tile([128, 128], f32)

        # ---- persistent tiles ----
        # norm bank: cols 0..3 am-norm^2 A-blocks, 4..7 B-blocks, 8+t hvs
        # norm^2 of tile t. NOTE: opening a multi-matmul accumulation group
        # (start=True, stop=False) resets the whole PSUM bank row, wiping
        # co-resident accumulators — so this bank is zeroed once and ALL its
        # groups accumulate with start=False.
        nrm = nrm_p.tile([128, 8 + NT], f32)
        nc.vector.memset(nrm[:], 0.0)
        preds_acc = acc_p.tile([128, NT], u32)
        eta_acc = acc_p.tile([128, NT], f32)
        mx_acc = acc_p.tile([128, NT], f32)          # A-half max per tile
        inv_cb = acc_p.tile([128, C], f32)           # bcast 1/am-norm

        am_tiles = {}   # (half, k) -> [128, CH] tile
        hx_tiles = {}   # t -> [128, D] tile

        def dma_am(half, k):
            t = am_p.tile([128, CH], f32r, tag=f"am{half}_{k}")
            nc.sync.dma_start(t[:], amT[k * 128:(k + 1) * 128,
                                        half * CH:(half + 1) * CH])
            am_tiles[(half, k)] = t

        def _hx_views(t):
            if t not in hx_tiles:
                hx_tiles[t] = hx_p.tile([128, D], f32r, tag="hx",
                                        name=f"hx{t}")
            src = hvsT[:, t * 128:(t + 1) * 128].rearrange(
                "(dc p) j -> p dc j", p=128)
            hxv = hx_tiles[t][:].rearrange("p (dc j) -> p dc j", j=128)
            return hxv, src

        def dma_hx_quarter(t, q):
            hxv, src = _hx_views(t)
            qc = DCH // 4
            nc.sync.dma_start(hxv[:, q * qc:(q + 1) * qc, :],
                              src[:, q * qc:(q + 1) * qc, :])



        # per-tile wave state
        wv_tile = {}    # (t, half) -> psum tile

        def sims_mm(t, half, k):
            key = (t, half)
            if key not in wv_tile:
                wv_tile[key] = wv_p.tile([128, CH], f32, tag="wv",
                                         name=f"wv{t}_{half}")
            nc.tensor.matmul(wv_tile[key][:],
                             hx_tiles[t][:, k * 128:(k + 1) * 128],
                             am_tiles[(half, k)][:],
                             start=(k == 0), stop=(k == DCH - 1))

        def hxsq_piece(t, g):
            """ACT: square 4 chunks of hx[t] (chunks 4g..4g+3) to bf16."""
            p = sqh_p.tile([128, 512], bf16, tag="sqh", name=f"sqh{t}_{g}")
            nc.scalar.square(p[:], hx_tiles[t][:, g * 512:(g + 1) * 512]
                             .bitcast(f32))
            return p

        def hvs_norm_mm(t, k, piece):
            nc.tensor.matmul(nrm[:, 8 + t:9 + t],
                             piece[:, (k % 4) * 128:(k % 4 + 1) * 128],
                             ones_t[:], start=False, stop=(k == DCH - 1),
                             skip_group_check=True)

        def am_norm_sq(half, k):
            """Square of am chunk (bf16). Window A uses DVE (idle then);
            window B uses ACT because DVE is backlogged with A epilogues."""
            sq = sqa_p.tile([128, CH], bf16, tag="sqa", name=f"sqa{half}_{k}")
            if half == 0:
                nc.vector.tensor_mul(sq[:], am_tiles[(half, k)][:]
                                     .bitcast(f32),
                                     am_tiles[(half, k)][:].bitcast(f32))
            else:
                nc.scalar.square(sq[:], am_tiles[(half, k)][:].bitcast(f32))
            return sq

        def am_norm_mms(half, k, sq):
            """4 reversed norm matmuls (ap=1, ~0 PE engine time)."""
            for b in range(4):
                nc.tensor.matmul(nrm[:, 4 * half + b:4 * half + b + 1],
                                 sq[:, b * 128:(b + 1) * 128], ones_t[:],
                                 start=False, stop=(k == DCH - 1),
                                 skip_group_check=True)

        def am_norm_finalize(half):
            """norm^2 cols [c,1] -> inv_cb[:, half*CH:(half+1)*CH]."""
            amn = ep_p.tile([128, 4], f32, tag="amn", name=f"amn{half}")
            nc.scalar.sqrt(amn[:], nrm[:, 4 * half:4 * half + 4])
            nc.vector.tensor_scalar_max(amn[:], amn[:], EPS)
            invq = ep_p.tile([128, 4], f32, tag="invq", name=f"invq{half}")
            nc.vector.reciprocal(invq[:], amn[:])
            # transpose each column [128c, 1] -> [1, 128c] via matmul vs
            # identity, landing all 4 blocks in one [1, CH] row
            trp = trbc_p.tile([1, CH], f32, tag="trbc", name=f"tr{half}")
            for b in range(4):
                nc.tensor.matmul(trp[:, b * 128:(b + 1) * 128],
                                 invq[:, b:b + 1], ident_t[:],
                                 start=True, stop=True)
            # borrows an sc-pool slot ([1,CH] fits the [128,CH] slot) to
            # avoid a dedicated 2KB/partition allocation
            inv_row = sc_p.tile([1, CH], f32, tag="sc",
                                name=f"invrow{half}", bufs=2)
            nc.scalar.copy(inv_row[:], trp[:])
            # exact fp32 ones outer-product broadcast to all partitions
            bcp = trbc_p.tile([128, CH], f32, tag="trbc", name=f"bc{half}")
            nc.tensor.matmul(bcp[:], ones_ct[:], inv_row[:],
                             start=True, stop=True)
            nc.scalar.copy(inv_cb[:, half * CH:(half + 1) * CH], bcp[:])

        def scale_A(t):
            """The bank-freeing scale; split out so the wave can run all
            four muls before any argmax work (the sc tag is 4 bufs deep)."""
            wv = wv_tile.pop((t, 0))
            sc = sc_p.tile([128, CH], f32, tag="sc", name=f"scA{t}",
                           bufs=2)
            nc.vector.tensor_mul(sc[:], wv[:], inv_cb[:, 0:CH])
            return sc

        def argmax_A(t, sc):
            mx8 = ep_p.tile([128, 8], f32, tag="mx8", name=f"mx8{t}")
            nc.vector.max(out=mx8[:], in_=sc[:])
            ix = ep_p.tile([128, 8], u32, tag="ix", name=f"ixA{t}")
            nc.vector.max_index(out=ix[:], in_max=mx8[:], in_values=sc[:])
            nc.vector.tensor_copy(mx_acc[:, t:t + 1], mx8[:, 0:1])
            nc.vector.tensor_copy(preds_acc[:, t:t + 1], ix[:, 0:1])
            # stash sims[:,1]-sims[:,0] in eta's slot; eta_fin finishes it
            # in place
            nc.vector.tensor_sub(eta_acc[:, t:t + 1], sc[:, 1:2], sc[:, 0:1])

        def eta_fin(t):
            nrm_t = ep_p.tile([128, 1], f32, tag="nrmt", name=f"nrmt{t}")
            nc.scalar.sqrt(nrm_t[:], nrm[:, 8 + t:9 + t])
            nc.vector.tensor_scalar_max(nrm_t[:], nrm_t[:], EPS)
            inv_n = ep_p.tile([128, 1], f32, tag="invn", name=f"invn{t}")
            nc.vector.reciprocal(inv_n[:], nrm_t[:])
            ec = eta_acc[:, t:t + 1]
            nc.vector.tensor_mul(ec, ec, inv_n[:])
            nc.vector.tensor_scalar(
                out=ec, in0=ec, scalar1=0.25, scalar2=0.5,
                op0=mybir.AluOpType.mult, op1=mybir.AluOpType.add)

        def epilogue_A(t):
            argmax_A(t, scale_A(t))
            eta_fin(t)

        def epilogue_B(t):
            wv = wv_tile.pop((t, 1))
            sc = sc_p.tile([128, CH], f32, tag="sc", name=f"scB{t}",
                           bufs=2)
            nc.vector.tensor_mul(sc[:], wv[:], inv_cb[:, CH:C])
            mxB = ep_p.tile([128, 8], f32, tag="mxB", name=f"mxB{t}")
            nc.vector.max(out=mxB[:], in_=sc[:])
            ix = ep_p.tile([128, 8], u32, tag="ix", name=f"ixB{t}")
            nc.vector.max_index(out=ix[:], in_max=mxB[:], in_values=sc[:])
            ixb = ep_p.tile([128, 1], u32, tag="ixb", name=f"ixb{t}")
            nc.vector.tensor_scalar_add(ixb[:], ix[:, 0:1], CH)
            mask = ep_p.tile([128, 1], u32, tag="mask", name=f"mask{t}")
            nc.vector.tensor_tensor(mask[:], mxB[:, 0:1],
                                    mx_acc[:, t:t + 1],
                                    mybir.AluOpType.is_gt)
            nc.vector.copy_predicated(preds_acc[:, t:t + 1], mask[:], ixb[:])

        # ================= emission =================
        # ---- window A: interleave hx quarters + amA chunks; wave tiles
        # accumulate chunk-paced with catch-up as their hx arrives ----
        emitted_pieces = {}

        def emit_wave_chunk(t, k):
            g = k // 4
            if (t, g) not in emitted_pieces:
                emitted_pieces[(t, g)] = hxsq_piece(t, g)
            sims_mm(t, 0, k)
            hvs_norm_mm(t, k, emitted_pieces[(t, g)])

        for k in range(DCH):
            if k < 16:
                # quarter q of tile t goes out before amA chunk 4q+t
                dma_hx_quarter(k % 4, k // 4)
            dma_am(0, k)
            if k == 0:
                nc.sync.dma_start(ones_t[:], ones_b[:])
            if k == 16:
                # constants needed only by am_norm_finalize; DMA'd here so
                # they don't delay the hx/am prefix
                nc.sync.dma_start(ones_ct[:], ones_r[:])
                nc.sync.dma_start(ident_t[:], ident[:])
            sq = am_norm_sq(0, k)
            # tile t joins at chunk t (its first hx quarter has landed)
            for t in range(WAVE):
                if t == k:
                    for kk in range(k + 1):
                        emit_wave_chunk(t, kk)
                elif t < k:
                    emit_wave_chunk(t, k)
            am_norm_mms(0, k, sq)

        # ---- A finalize + wave A epilogues: run three bank-freeing muls
        # before any argmax work (sc tag is 3 bufs deep) ----
        am_norm_finalize(0)
        sc0 = scale_A(0)
        sc1 = scale_A(1)
        argmax_A(0, sc0)
        sc2 = scale_A(2)
        argmax_A(1, sc1)
        sc3 = scale_A(3)
        argmax_A(2, sc2)
        argmax_A(3, sc3)
        for t in range(WAVE):
            eta_fin(t)

        # ---- window B: amB chunks, wave B halves chunk-paced ----
        for k in range(DCH):
            dma_am(1, k)
            sq = am_norm_sq(1, k)
            for t in range(WAVE):
                sims_mm(t, 1, k)
            am_norm_mms(1, k, sq)

        # prefetch next hx tiles (quarters so the first chunks land ~1.5us
        # after the reused slot frees, instead of a full 6us tile)
        for nt in range(4, 8):
            for q in range(4):
                dma_hx_quarter(nt, q)

        am_norm_finalize(1)
        for t in range(WAVE):
            epilogue_B(t)

        # ---- steady tiles ----
        for t in range(WAVE, NT):
            # emit all squares upfront so ACT runs ahead of the PE norm mms
            # (the 4-deep piece pool throttles ACT's lookahead via WARs)
            for g in range(8):
                emitted_pieces[(t, g)] = hxsq_piece(t, g)
            for k in range(DCH):
                sims_mm(t, 0, k)
                hvs_norm_mm(t, k, emitted_pieces[(t, k // 4)])
            epilogue_A(t)
            for k in range(DCH):
                sims_mm(t, 1, k)
            epilogue_B(t)
            # prefetch 4 tiles ahead (slot distance == pool bufs, so the
            # WAR lands after this tile's last hx read)
            if t + 4 < NT:
                for q in range(4):
                    dma_hx_quarter(t + 4, q)
            if t == 7:
                # first half of the outputs is final; overlap its DMA
                nc.sync.dma_start(preds_o[:, 0:8], preds_acc[:, 0:8])
                nc.sync.dma_start(eta_o[:, 0:8], eta_acc[:, 0:8])

        nc.sync.dma_start(preds_o[:, 8:NT], preds_acc[:, 8:NT])
        nc.sync.dma_start(eta_o[:, 8:NT], eta_acc[:, 8:NT])

    _split_multiwait(nc)
    return nc


_CACHE = {}


def kernel(hvs: np.ndarray, am: np.ndarray):
    hvs = np.asarray(hvs, dtype=np.float32)
    am = np.asarray(am, dtype=np.float32)
    assert hvs.shape == (N_FULL, D) and am.shape == (C, D)

    if "nc" not in _CACHE:
        _CACHE["nc"] = build_nc()
    nc = _CACHE["nc"]

    amT = np.ascontiguousarray(am.T)                      # [D, C]
    import ml_dtypes
    ones_b = np.ones((128, 1), dtype=ml_dtypes.bfloat16)
    ones_r = np.ones((1, 128), dtype=np.float32)
    ident = np.eye(128, dtype=np.float32)

    in_maps = []
    for r in range(N_CORES):
        shard = hvs[r * NS:(r + 1) * NS]                  # [NS, D]
        hvsT = np.ascontiguousarray(shard.T)              # [D, NS]
        in_maps.append({"hvsT": hvsT, "amT": amT, "ones_b": ones_b,
                        "ones_r": ones_r, "ident": ident})

    res = run_bass_kernel_spmd(nc, in_maps, core_ids=list(range(N_CORES)))

    preds = np.empty(N_FULL, dtype=np.int32)
    eta = np.empty(N_FULL, dtype=np.float32)
    for r in range(N_CORES):
        p = res.results[r]["preds"]                       # [128, NT] u32
        e = res.results[r]["eta"]                         # [128, NT] f32
        preds[r * NS:(r + 1) * NS] = p.T.ravel().astype(np.int32)
        eta[r * NS:(r + 1) * NS] = e.T.ravel()
    return preds, eta
